# revision 4
# baseline (speedup 1.0000x reference)
"""Trainium2 Bass kernel for nn_CNNFusing (session attention pooling), v2.

Math (per session s of L=50 tokens, H=128):
  hidden = max(intra, inter)                                 [T, H]
  v_n[s] = hidden[last token of s]                           [B, H]
  y[t]   = W1 @ v_n[s(t)] + W2 @ hidden[t] + (b1 + b2)       [T, H]
  alpha[t] = q_w . sigmoid(y[t]) + q_b                       [T]
  s_g[s] = sum_{t in s} alpha[t] * hidden[t]                 [B, H]
  out[s] = [v_n[s], s_g[s]] @ W3.T + b3                      [B, H]

v2 changes vs baseline:
  - The elementwise max runs inside the DMA: SWDGE loads intra with an
    fp32->bf16 cast, then loads inter into the same tile with
    accum_op=max (CCE inline max).  No DVE max op at all.
  - bf16 everywhere on-chip (abs tolerance is ~0.55, enormous margin):
    transposed hidden (ht), sigmoid output, weights, v_n, s_g, final
    matmuls.  Halves SBUF traffic and enables FWL weight loads.
  - PSUM double-buffered for the transpose staging tile (ps_t bufs=2),
    so PE transposes ping-pong across banks instead of waiting for the
    PSUM->SBUF copy each chunk.
  - W2 / W1 / q_w matmuls batched per 1280-col group so LDWEIGHTS swaps
    3x per group instead of per 512-piece.
  - ht PSUM->SBUF copies split between ACT and DVE.
  - Final W3 matmuls run per-macro (the macro IS one 128-session block)
    instead of in a tail phase.
"""

import numpy as np

H = 128
L = 50
N_CORES = 8
MACRO = 6400          # tokens per macro-tile = 128 sessions
NCB = MACRO // 128    # 50 position blocks per macro
GRP = 1024            # tokens per matmul group (2 fp32 PSUM banks)
PIECE = 512           # matmul piece (1 fp32 PSUM bank)
# groups per macro: six of 1024 plus a 256 tail
GROUPS = [(t0, min(GRP, MACRO - t0)) for t0 in range(0, MACRO, GRP)]

_cache: dict = {}


def _bf16_dtype():
    import concourse.mybir as mybir
    return mybir.dt.np(mybir.dt.bfloat16)


def _numpy_ref(intra_item_emb, inter_item_emb, W1, b1, W2, b2, q_w, q_b, W3, b3,
               seq_len):
    hidden = np.maximum(intra_item_emb, inter_item_emb)
    nB = seq_len.shape[0]
    seg_ids = np.repeat(np.arange(nB), seq_len)
    last_idx = np.cumsum(seq_len) - 1
    v_n = hidden[last_idx]
    v_n_rep = v_n[seg_ids]
    z = v_n_rep @ W1.T + b1 + hidden @ W2.T + b2
    alpha = (1.0 / (1.0 + np.exp(-z))) @ q_w.T + q_b
    s_g = np.zeros((nB, hidden.shape[1]), np.float32)
    np.add.at(s_g, seg_ids, alpha * hidden)
    return (np.concatenate([v_n, s_g], axis=1) @ W3.T + b3).astype(np.float32)


def _build(t_core: int, q_b_val: float, loop_reps: int | None = None):
    """Build the per-core Bass program. t_core tokens (multiple of MACRO)."""
    import concourse.mybir as mybir
    import concourse.tile as tile
    from concourse import bacc
    from concourse.masks import make_identity

    f32 = mybir.dt.float32
    bf16 = mybir.dt.bfloat16

    n_macro = t_core // MACRO
    assert t_core % MACRO == 0
    b_core = t_core // L

    nc = bacc.Bacc(trn_type="TRN2", num_devices=N_CORES)

    intra = nc.dram_tensor("intra", [t_core, H], f32, kind="ExternalInput").ap()
    inter = nc.dram_tensor("inter", [t_core, H], f32, kind="ExternalInput").ap()
    w1t_d = nc.dram_tensor("w1t", [H, H], bf16, kind="ExternalInput").ap()
    w2t_d = nc.dram_tensor("w2t", [H, H], bf16, kind="ExternalInput").ap()
    qwbc_d = nc.dram_tensor("qwbc", [H, H], bf16, kind="ExternalInput").ap()
    b12_d = nc.dram_tensor("b12", [H, 1], f32, kind="ExternalInput").ap()
    w3at_d = nc.dram_tensor("w3at", [H, H], bf16, kind="ExternalInput").ap()
    w3bt_d = nc.dram_tensor("w3bt", [H, H], bf16, kind="ExternalInput").ap()
    b3r_d = nc.dram_tensor("b3r", [1, H], bf16, kind="ExternalInput").ap()
    out_d = nc.dram_tensor("h_s", [b_core, H], f32, kind="ExternalOutput").ap()

    # token t = m*MACRO + 50*p + c: partition p = session, position c
    intra_r = intra.rearrange("(m p c) h -> m p c h", p=128, c=L)
    inter_r = inter.rearrange("(m p c) h -> m p c h", p=128, c=L)

    with tile.TileContext(nc) as tc:
        with (
            tc.tile_pool(name="consts", bufs=1) as consts,
            tc.tile_pool(name="inp", bufs=2) as inp,
            tc.tile_pool(name="hts", bufs=2) as hts,
            tc.tile_pool(name="sig", bufs=2) as sig,
            tc.tile_pool(name="wts", bufs=2) as wts,
            tc.tile_pool(name="tmps", bufs=2) as tmps,
            tc.tile_pool(name="pers", bufs=1) as pers,
            tc.tile_pool(name="ps_t", bufs=2, space="PSUM") as ps_t,
            tc.tile_pool(name="ps_y", bufs=2, space="PSUM") as ps_y,
            tc.tile_pool(name="ps_f", bufs=1, space="PSUM") as ps_f,
        ):
            w1t = consts.tile([H, H], bf16)
            nc.sync.dma_start(w1t, w1t_d)
            w2t = consts.tile([H, H], bf16)
            nc.sync.dma_start(w2t, w2t_d)
            qwbc = consts.tile([H, H], bf16)
            nc.sync.dma_start(qwbc, qwbc_d)
            b12 = consts.tile([H, 1], f32)
            nc.sync.dma_start(b12, b12_d)
            w3at = consts.tile([H, H], bf16)
            nc.sync.dma_start(w3at, w3at_d)
            w3bt = consts.tile([H, H], bf16)
            nc.sync.dma_start(w3bt, w3bt_d)
            b3r = consts.tile([1, H], bf16)
            nc.sync.dma_start(b3r, b3r_d)
            ident = consts.tile([H, H], bf16)
            make_identity(nc, ident)
            ones1 = consts.tile([1, H], bf16)
            nc.vector.memset(ones1, 1.0)

            v_nt = pers.tile([H, b_core], bf16)   # [h, session]
            s_gt = pers.tile([H, b_core], bf16)   # [h, session]
            hs_sb = pers.tile([128, n_macro, H], f32)

            if loop_reps is not None:
                _loop_cm = tc.For_i(0, loop_reps, 1)
                _loop_cm.__enter__()
            else:
                _loop_cm = None

            for m in range(n_macro):
                mb = slice(m * 128, (m + 1) * 128)
                # SWDGE cast-DMAs (fp32 HBM -> bf16 SBUF), then a bf16 DVE
                # max (2x mode).  Split in position halves for finer
                # pipelining.
                ia = inp.tile([128, L, H], bf16, tag="ia")
                ib = inp.tile([128, L, H], bf16, tag="ib")
                hd = inp.tile([128, L, H], bf16, tag="hd")
                for x in range(2):
                    sl = slice(x * 25, (x + 1) * 25)
                    nc.gpsimd.dma_start(ia[:, sl, :], intra_r[m, :, sl])
                    nc.gpsimd.dma_start(ib[:, sl, :], inter_r[m, :, sl])
                for x in range(2):
                    sl = slice(x * 25, (x + 1) * 25)
                    nc.vector.tensor_tensor(hd[:, sl, :], ia[:, sl, :],
                                            ib[:, sl, :], mybir.AluOpType.max)

                # transpose to [h, t']; column cg*128+p = (session p, pos cg)
                # tp=9 chunk (positions 45-49) first so v_n is ready early
                ht = hts.tile([H, MACRO], bf16, tag="ht")
                for i, tp in enumerate([9] + list(range(9))):
                    pt = ps_t.tile([128, 640], bf16, tag="pt")
                    for k in range(5):
                        cg = tp * 5 + k
                        nc.tensor.transpose(
                            pt[:, k * 128:(k + 1) * 128], hd[:, cg, :], ident)
                    if tp == 9:
                        nc.scalar.copy(v_nt[:, mb], pt[:, 4 * 128:5 * 128])
                    # split PSUM->SBUF copies between ACT and DVE
                    dst = ht[:, tp * 640:(tp + 1) * 640]
                    if i % 2 == 0:
                        nc.scalar.copy(dst, pt)
                    else:
                        nc.vector.tensor_copy(dst, pt)

                wt = wts.tile([H, MACRO], bf16, tag="wt")
                vb = v_nt[:, mb]
                for (t0, gsz) in GROUPS:
                    pieces = [(a, min(a + PIECE, gsz)) for a in range(0, gsz, PIECE)]
                    py_full = ps_y.tile([128, GRP], f32, tag="py")
                    py = py_full[:, :gsz]
                    # W2 on all pieces (one LDWEIGHTS), then W1 broadcast
                    for (a, b) in pieces:
                        nc.tensor.matmul(py[:, a:b], lhsT=w2t,
                                         rhs=ht[:, t0 + a:t0 + b],
                                         start=True, stop=False)
                    for (a, b) in pieces:
                        u_p = vb[:, None, :].to_broadcast((H, (b - a) // 128, 128))
                        nc.tensor.matmul(py[:, a:b], lhsT=w1t, rhs=u_p,
                                         start=False, stop=True)
                    st_full = sig.tile([H, GRP], bf16, tag="st")
                    st = st_full[:, :gsz]
                    nc.scalar.activation(
                        st, py, mybir.ActivationFunctionType.Sigmoid,
                        bias=b12)
                    for (a, b) in pieces:
                        nc.tensor.matmul(py[:, a:b], lhsT=qwbc,
                                         rhs=st[:, a:b],
                                         start=True, stop=True)
                    # wt = (alpha_tilde + q_b) * hT
                    nc.vector.scalar_tensor_tensor(
                        out=wt[:, t0:t0 + gsz], in0=py,
                        scalar=float(q_b_val),
                        in1=ht[:, t0:t0 + gsz],
                        op0=mybir.AluOpType.add, op1=mybir.AluOpType.mult)

                # segment sum: pairwise halving tree over the 50 position
                # blocks (session p = column p of each block).  First level
                # (the big half) on DVE at bf16 2x; the rest on GpSimd.
                wtv = wt.rearrange("h (c s) -> h c s", s=128)
                tm = tmps.tile([H, NCB // 2, 128], bf16, tag="tm")
                nc.vector.tensor_tensor(
                    tm, wtv[:, 0:25], wtv[:, 25:50], mybir.AluOpType.add)
                n = NCB // 2
                while n > 1:
                    if n % 2:
                        nc.gpsimd.tensor_tensor(
                            tm[:, 0], tm[:, 0], tm[:, n - 1],
                            mybir.AluOpType.add)
                        n -= 1
                    k = n // 2
                    nc.gpsimd.tensor_tensor(
                        tm[:, 0:k], tm[:, 0:k], tm[:, k:2 * k],
                        mybir.AluOpType.add)
                    n = k
                nc.gpsimd.tensor_copy(out=s_gt[:, mb], in_=tm[:, 0])

                # final: out[s, :] = v_n W3a^T + s_g W3b^T + b3 for this block
                # (own PSUM pool so holding it until the tree finishes does
                # not block the next macro's y-group PSUM ring)
                pf = ps_f.tile([128, H], f32, tag="pf", name="pf")
                nc.tensor.matmul(pf, lhsT=v_nt[:, mb], rhs=w3at,
                                 start=True, stop=False)
                nc.tensor.matmul(pf, lhsT=s_gt[:, mb], rhs=w3bt,
                                 start=False, stop=False)
                nc.tensor.matmul(pf, lhsT=ones1, rhs=b3r,
                                 start=False, stop=True)
                nc.vector.tensor_copy(hs_sb[:, m, :], pf)

            nc.sync.dma_start(out_d.rearrange("(g p) h -> p g h", p=128), hs_sb)

            if _loop_cm is not None:
                _loop_cm.__exit__(None, None, None)

    nc.compile()
    return nc


def _prep_weights(W1, b1, W2, b2, q_w, q_b, W3, b3):
    bf = _bf16_dtype()
    w1t = np.ascontiguousarray(W1.T).astype(bf)
    w2t = np.ascontiguousarray(W2.T).astype(bf)
    qwbc = np.ascontiguousarray(
        np.repeat(q_w.reshape(H, 1), H, axis=1)).astype(bf)
    b12 = np.ascontiguousarray((b1 + b2).reshape(H, 1)).astype(np.float32)
    w3at = np.ascontiguousarray(W3[:, :H].T).astype(bf)
    w3bt = np.ascontiguousarray(W3[:, H:].T).astype(bf)
    b3r = np.ascontiguousarray(b3.reshape(1, H)).astype(bf)
    return dict(w1t=w1t, w2t=w2t, qwbc=qwbc, b12=b12, w3at=w3at, w3bt=w3bt,
                b3r=b3r)


def kernel(intra_item_emb, inter_item_emb, W1, b1, W2, b2, q_w, q_b, W3, b3,
           seq_len):
    intra_item_emb = np.ascontiguousarray(np.asarray(intra_item_emb, np.float32))
    inter_item_emb = np.ascontiguousarray(np.asarray(inter_item_emb, np.float32))
    W1 = np.asarray(W1, np.float32)
    b1 = np.asarray(b1, np.float32)
    W2 = np.asarray(W2, np.float32)
    b2 = np.asarray(b2, np.float32)
    q_w = np.asarray(q_w, np.float32)
    q_b = np.asarray(q_b, np.float32)
    W3 = np.asarray(W3, np.float32)
    b3 = np.asarray(b3, np.float32)
    seq_len = np.asarray(seq_len)

    T, h = intra_item_emb.shape
    B = seq_len.shape[0]
    if (h != H or not np.all(seq_len == L) or T != B * L
            or T % (N_CORES * MACRO) != 0):
        return _numpy_ref(intra_item_emb, inter_item_emb, W1, b1, W2, b2, q_w,
                          q_b, W3, b3, seq_len)

    from concourse.bass_utils import run_bass_kernel_spmd

    t_core = T // N_CORES
    key = (t_core, float(q_b[0]))
    if key not in _cache:
        _cache.clear()
        _cache[key] = _build(t_core, float(q_b[0]))
    nc = _cache[key]

    wts = _prep_weights(W1, b1, W2, b2, q_w, q_b, W3, b3)
    in_maps = []
    for c in range(N_CORES):
        sl = slice(c * t_core, (c + 1) * t_core)
        m = {"intra": intra_item_emb[sl], "inter": inter_item_emb[sl]}
        m.update(wts)
        in_maps.append(m)

    res = run_bass_kernel_spmd(nc, in_maps, core_ids=list(range(N_CORES)))
    return np.concatenate([res.results[c]["h_s"] for c in range(N_CORES)],
                          axis=0)


# revision 5
# speedup vs baseline: 2.2082x; 2.2082x over previous
"""Trainium2 Bass kernel for nn_CNNFusing (session attention pooling), v2.

Math (per session s of L=50 tokens, H=128):
  hidden = max(intra, inter)                                 [T, H]
  v_n[s] = hidden[last token of s]                           [B, H]
  y[t]   = W1 @ v_n[s(t)] + W2 @ hidden[t] + (b1 + b2)       [T, H]
  alpha[t] = q_w . sigmoid(y[t]) + q_b                       [T]
  s_g[s] = sum_{t in s} alpha[t] * hidden[t]                 [B, H]
  out[s] = [v_n[s], s_g[s]] @ W3.T + b3                      [B, H]

v2 changes vs baseline:
  - The elementwise max runs inside the DMA: SWDGE loads intra with an
    fp32->bf16 cast, then loads inter into the same tile with
    accum_op=max (CCE inline max).  No DVE max op at all.
  - bf16 everywhere on-chip (abs tolerance is ~0.55, enormous margin):
    transposed hidden (ht), sigmoid output, weights, v_n, s_g, final
    matmuls.  Halves SBUF traffic and enables FWL weight loads.
  - PSUM double-buffered for the transpose staging tile (ps_t bufs=2),
    so PE transposes ping-pong across banks instead of waiting for the
    PSUM->SBUF copy each chunk.
  - W2 / W1 / q_w matmuls batched per 1280-col group so LDWEIGHTS swaps
    3x per group instead of per 512-piece.
  - ht PSUM->SBUF copies split between ACT and DVE.
  - Final W3 matmuls run per-macro (the macro IS one 128-session block)
    instead of in a tail phase.
"""

import numpy as np

H = 128
L = 50
N_CORES = 8
MACRO = 6400          # tokens per macro-tile = 128 sessions
NCB = MACRO // 128    # 50 position blocks per macro
GRP = 1024            # tokens per matmul group (2 fp32 PSUM banks)
PIECE = 512           # matmul piece (1 fp32 PSUM bank)
# groups per macro: six of 1024 plus a 256 tail
GROUPS = [(t0, min(GRP, MACRO - t0)) for t0 in range(0, MACRO, GRP)]

_cache: dict = {}


def _bf16_dtype():
    import concourse.mybir as mybir
    return mybir.dt.np(mybir.dt.bfloat16)


def _numpy_ref(intra_item_emb, inter_item_emb, W1, b1, W2, b2, q_w, q_b, W3, b3,
               seq_len):
    hidden = np.maximum(intra_item_emb, inter_item_emb)
    nB = seq_len.shape[0]
    seg_ids = np.repeat(np.arange(nB), seq_len)
    last_idx = np.cumsum(seq_len) - 1
    v_n = hidden[last_idx]
    v_n_rep = v_n[seg_ids]
    z = v_n_rep @ W1.T + b1 + hidden @ W2.T + b2
    alpha = (1.0 / (1.0 + np.exp(-z))) @ q_w.T + q_b
    s_g = np.zeros((nB, hidden.shape[1]), np.float32)
    np.add.at(s_g, seg_ids, alpha * hidden)
    return (np.concatenate([v_n, s_g], axis=1) @ W3.T + b3).astype(np.float32)


def _build(t_core: int, q_b_val: float, loop_reps: int | None = None):
    """Build the per-core Bass program. t_core tokens (multiple of MACRO)."""
    import concourse.mybir as mybir
    import concourse.tile as tile
    from concourse import bacc
    from concourse.masks import make_identity

    f32 = mybir.dt.float32
    bf16 = mybir.dt.bfloat16

    n_macro = t_core // MACRO
    assert t_core % MACRO == 0
    b_core = t_core // L

    nc = bacc.Bacc(trn_type="TRN2", num_devices=N_CORES)

    intra = nc.dram_tensor("intra", [t_core, H], f32, kind="ExternalInput").ap()
    inter = nc.dram_tensor("inter", [t_core, H], f32, kind="ExternalInput").ap()
    w1t_d = nc.dram_tensor("w1t", [H, H], bf16, kind="ExternalInput").ap()
    w2t_d = nc.dram_tensor("w2t", [H, H], bf16, kind="ExternalInput").ap()
    qwbc_d = nc.dram_tensor("qwbc", [H, H], bf16, kind="ExternalInput").ap()
    b12_d = nc.dram_tensor("b12", [H, 1], f32, kind="ExternalInput").ap()
    w3at_d = nc.dram_tensor("w3at", [H, H], bf16, kind="ExternalInput").ap()
    w3bt_d = nc.dram_tensor("w3bt", [H, H], bf16, kind="ExternalInput").ap()
    b3r_d = nc.dram_tensor("b3r", [1, H], bf16, kind="ExternalInput").ap()
    out_d = nc.dram_tensor("h_s", [b_core, H], f32, kind="ExternalOutput").ap()

    # token t = m*MACRO + 50*p + c: partition p = session, position c
    intra_r = intra.rearrange("(m p c) h -> m p c h", p=128, c=L)
    inter_r = inter.rearrange("(m p c) h -> m p c h", p=128, c=L)

    with tile.TileContext(nc) as tc:
        with (
            tc.tile_pool(name="consts", bufs=1) as consts,
            tc.tile_pool(name="inp", bufs=2) as inp,
            tc.tile_pool(name="hts", bufs=2) as hts,
            tc.tile_pool(name="sig", bufs=2) as sig,
            tc.tile_pool(name="wts", bufs=2) as wts,
            tc.tile_pool(name="tmps", bufs=2) as tmps,
            tc.tile_pool(name="pers", bufs=1) as pers,
            tc.tile_pool(name="ps_t", bufs=2, space="PSUM") as ps_t,
            tc.tile_pool(name="ps_y", bufs=2, space="PSUM") as ps_y,
            tc.tile_pool(name="ps_f", bufs=1, space="PSUM") as ps_f,
        ):
            w1t = consts.tile([H, H], bf16)
            nc.sync.dma_start(w1t, w1t_d)
            w2t = consts.tile([H, H], bf16)
            nc.sync.dma_start(w2t, w2t_d)
            qwbc = consts.tile([H, H], bf16)
            nc.sync.dma_start(qwbc, qwbc_d)
            b12 = consts.tile([H, 1], f32)
            nc.sync.dma_start(b12, b12_d)
            w3at = consts.tile([H, H], bf16)
            nc.sync.dma_start(w3at, w3at_d)
            w3bt = consts.tile([H, H], bf16)
            nc.sync.dma_start(w3bt, w3bt_d)
            b3r = consts.tile([1, H], bf16)
            nc.sync.dma_start(b3r, b3r_d)
            ident = consts.tile([H, H], bf16)
            make_identity(nc, ident)
            ones1 = consts.tile([1, H], bf16)
            nc.vector.memset(ones1, 1.0)
            qbc = consts.tile([H, 1], f32)
            nc.vector.memset(qbc, float(q_b_val))

            v_nt = pers.tile([H, b_core], bf16)   # [h, session]
            s_gt = pers.tile([H, b_core], bf16)   # [h, session]
            hs_sb = pers.tile([128, n_macro, H], f32)

            if loop_reps is not None:
                _loop_cm = tc.For_i(0, loop_reps, 1)
                _loop_cm.__enter__()
            else:
                _loop_cm = None

            for m in range(n_macro):
                mb = slice(m * 128, (m + 1) * 128)
                # SWDGE cast-DMAs (fp32 HBM -> bf16 SBUF), then a bf16 DVE
                # max (2x mode).  Split in position halves for finer
                # pipelining.
                ia = inp.tile([128, L, H], bf16, tag="ia")
                ib = inp.tile([128, L, H], bf16, tag="ib")
                hd = inp.tile([128, L, H], bf16, tag="hd")
                nc.gpsimd.dma_start(ia, intra_r[m])
                nc.gpsimd.dma_start(ib, inter_r[m])
                for x in range(2):
                    sl = slice(x * 25, (x + 1) * 25)
                    nc.vector.tensor_tensor(hd[:, sl, :], ia[:, sl, :],
                                            ib[:, sl, :], mybir.AluOpType.max)

                # transpose to [h, t']; column cg*128+p = (session p, pos cg)
                # tp=9 chunk (positions 45-49) first so v_n is ready early
                ht = hts.tile([H, MACRO], bf16, tag="ht")
                for i, tp in enumerate([9] + list(range(9))):
                    pt = ps_t.tile([128, 640], bf16, tag="pt")
                    for k in range(5):
                        cg = tp * 5 + k
                        nc.tensor.transpose(
                            pt[:, k * 128:(k + 1) * 128], hd[:, cg, :], ident)
                    if tp == 9:
                        nc.scalar.copy(v_nt[:, mb], pt[:, 4 * 128:5 * 128])
                    # all PSUM->SBUF copies on ACT (DVE is the ceiling)
                    nc.scalar.copy(ht[:, tp * 640:(tp + 1) * 640], pt)

                wt = wts.tile([H, MACRO], bf16, tag="wt")
                vb = v_nt[:, mb]
                for (t0, gsz) in GROUPS:
                    pieces = [(a, min(a + PIECE, gsz)) for a in range(0, gsz, PIECE)]
                    py_full = ps_y.tile([128, GRP], f32, tag="py")
                    py = py_full[:, :gsz]
                    # W2 on all pieces (one LDWEIGHTS), then W1 broadcast
                    for (a, b) in pieces:
                        nc.tensor.matmul(py[:, a:b], lhsT=w2t,
                                         rhs=ht[:, t0 + a:t0 + b],
                                         start=True, stop=False)
                    for (a, b) in pieces:
                        u_p = vb[:, None, :].to_broadcast((H, (b - a) // 128, 128))
                        nc.tensor.matmul(py[:, a:b], lhsT=w1t, rhs=u_p,
                                         start=False, stop=True)
                    st_full = sig.tile([H, GRP], bf16, tag="st")
                    st = st_full[:, :gsz]
                    nc.scalar.activation(
                        st, py, mybir.ActivationFunctionType.Sigmoid,
                        bias=b12)
                    for (a, b) in pieces:
                        nc.tensor.matmul(py[:, a:b], lhsT=qwbc,
                                         rhs=st[:, a:b],
                                         start=True, stop=True)
                    # wt = (alpha_tilde + q_b) * hT.  For two groups per
                    # macro, route the +q_b through ACT (bf16 copy-add) so
                    # the DVE multiply runs at 2x on all-SBUF bf16.
                    if t0 in (0, GRP):
                        al_full = sig.tile([H, GRP], bf16, tag="al")
                        al = al_full[:, :gsz]
                        nc.scalar.add(al, py, qbc)
                        nc.vector.tensor_tensor(
                            wt[:, t0:t0 + gsz], al, ht[:, t0:t0 + gsz],
                            mybir.AluOpType.mult)
                    else:
                        nc.vector.scalar_tensor_tensor(
                            out=wt[:, t0:t0 + gsz], in0=py,
                            scalar=float(q_b_val),
                            in1=ht[:, t0:t0 + gsz],
                            op0=mybir.AluOpType.add, op1=mybir.AluOpType.mult)

                # segment sum: pairwise halving tree over the 50 position
                # blocks (session p = column p of each block), entirely on
                # DVE at bf16 2x.  (GpSimd bf16 tensor_tensor is very slow
                # on HW, and fp32 GpSimd was the v2 bottleneck.)
                wtv = wt.rearrange("h (c s) -> h c s", s=128)
                tm = tmps.tile([H, NCB // 2, 128], bf16, tag="tm")
                nc.vector.tensor_tensor(
                    tm, wtv[:, 0:25], wtv[:, 25:50], mybir.AluOpType.add)
                n = NCB // 2
                while n > 1:
                    if n % 2:
                        nc.vector.tensor_tensor(
                            tm[:, 0], tm[:, 0], tm[:, n - 1],
                            mybir.AluOpType.add)
                        n -= 1
                    k = n // 2
                    nc.vector.tensor_tensor(
                        tm[:, 0:k], tm[:, 0:k], tm[:, k:2 * k],
                        mybir.AluOpType.add)
                    n = k
                nc.vector.tensor_copy(out=s_gt[:, mb], in_=tm[:, 0])

                # final: out[s, :] = v_n W3a^T + s_g W3b^T + b3 for this block
                # (own PSUM pool so holding it until the tree finishes does
                # not block the next macro's y-group PSUM ring)
                pf = ps_f.tile([128, H], f32, tag="pf", name="pf")
                nc.tensor.matmul(pf, lhsT=v_nt[:, mb], rhs=w3at,
                                 start=True, stop=False)
                nc.tensor.matmul(pf, lhsT=s_gt[:, mb], rhs=w3bt,
                                 start=False, stop=False)
                nc.tensor.matmul(pf, lhsT=ones1, rhs=b3r,
                                 start=False, stop=True)
                nc.vector.tensor_copy(hs_sb[:, m, :], pf)

            nc.sync.dma_start(out_d.rearrange("(g p) h -> p g h", p=128), hs_sb)

            if _loop_cm is not None:
                _loop_cm.__exit__(None, None, None)

    nc.compile()
    return nc


def _prep_weights(W1, b1, W2, b2, q_w, q_b, W3, b3):
    bf = _bf16_dtype()
    w1t = np.ascontiguousarray(W1.T).astype(bf)
    w2t = np.ascontiguousarray(W2.T).astype(bf)
    qwbc = np.ascontiguousarray(
        np.repeat(q_w.reshape(H, 1), H, axis=1)).astype(bf)
    b12 = np.ascontiguousarray((b1 + b2).reshape(H, 1)).astype(np.float32)
    w3at = np.ascontiguousarray(W3[:, :H].T).astype(bf)
    w3bt = np.ascontiguousarray(W3[:, H:].T).astype(bf)
    b3r = np.ascontiguousarray(b3.reshape(1, H)).astype(bf)
    return dict(w1t=w1t, w2t=w2t, qwbc=qwbc, b12=b12, w3at=w3at, w3bt=w3bt,
                b3r=b3r)


def kernel(intra_item_emb, inter_item_emb, W1, b1, W2, b2, q_w, q_b, W3, b3,
           seq_len):
    intra_item_emb = np.ascontiguousarray(np.asarray(intra_item_emb, np.float32))
    inter_item_emb = np.ascontiguousarray(np.asarray(inter_item_emb, np.float32))
    W1 = np.asarray(W1, np.float32)
    b1 = np.asarray(b1, np.float32)
    W2 = np.asarray(W2, np.float32)
    b2 = np.asarray(b2, np.float32)
    q_w = np.asarray(q_w, np.float32)
    q_b = np.asarray(q_b, np.float32)
    W3 = np.asarray(W3, np.float32)
    b3 = np.asarray(b3, np.float32)
    seq_len = np.asarray(seq_len)

    T, h = intra_item_emb.shape
    B = seq_len.shape[0]
    if (h != H or not np.all(seq_len == L) or T != B * L
            or T % (N_CORES * MACRO) != 0):
        return _numpy_ref(intra_item_emb, inter_item_emb, W1, b1, W2, b2, q_w,
                          q_b, W3, b3, seq_len)

    from concourse.bass_utils import run_bass_kernel_spmd

    t_core = T // N_CORES
    key = (t_core, float(q_b[0]))
    if key not in _cache:
        _cache.clear()
        _cache[key] = _build(t_core, float(q_b[0]))
    nc = _cache[key]

    wts = _prep_weights(W1, b1, W2, b2, q_w, q_b, W3, b3)
    in_maps = []
    for c in range(N_CORES):
        sl = slice(c * t_core, (c + 1) * t_core)
        m = {"intra": intra_item_emb[sl], "inter": inter_item_emb[sl]}
        m.update(wts)
        in_maps.append(m)

    res = run_bass_kernel_spmd(nc, in_maps, core_ids=list(range(N_CORES)))
    return np.concatenate([res.results[c]["h_s"] for c in range(N_CORES)],
                          axis=0)
